# revision 3
# baseline (speedup 1.0000x reference)
"""BitDelta linear on 8 TRN2 NeuronCores — v2.

C[b,s,o] = sum_i X[b,s,i] * (base[o,i] + (2*signs[o,i]-1)*scales[o])

Sharding: TG token-groups x FG feature-groups. The host side casts X to
bf16 and re-lays it out i-major (X^T) so the device does zero X
transposes; each token chunk is one contiguous DMA (32KB/partition
descriptors):

  xt[(c*128+p), k, t] = X[tg*T_C + c*chunk_t + t, k*128 + p]   (bf16)

On device the weight delta W = base + (2s-1)*scale is applied on DVE
(base arrives bf16, signs uint8 {0,1}, scales f32), W^T is built
k-major via identity-matmul on the PE once (startup only), and the
main loop is a pure MM stream: per 128-token tile, KT ldweights of the
stationary X^T tile + KT*FB matmuls of N=512 accumulating into PSUM,
then ACT/DVE copy PSUM->SBUF and DMA out.

Optionally flips walrus's --enable-ldw-opt to true (redundant
load-weight elimination) via LDW_OPT.
"""

import sys

sys.path.insert(0, "/opt/trn_rl_repo")

from contextlib import ExitStack

import numpy as np
import ml_dtypes

import concourse.bass as bass
import concourse.tile as tile
import concourse.bass_utils as _bu
from concourse import bacc, mybir
from concourse.alu_op_type import AluOpType
from concourse.masks import make_identity

F32 = mybir.dt.float32
BF16 = mybir.dt.bfloat16
U8 = mybir.dt.uint8
P = 128

B, S, IN, OUT = 8, 2048, 4096, 4096
T = B * S
N_CORES = 8

# ---- walrus flag patch: redundant-ldweights elimination ----
LDW_OPT = False
_orig_run_command = _bu.run_command


def _run_command_ldw(cmd, **kw):
    if LDW_OPT and isinstance(cmd, list):
        cmd = ["--enable-ldw-opt=true" if c == "--enable-ldw-opt=false" else c
               for c in cmd]
    return _orig_run_command(cmd, **kw)


_bu.run_command = _run_command_ldw


def _fuse_ldweights(nc):
    """Fold standalone InstLdweights into their InstMatmult (self-loading),
    transferring sync waits/updates. Lets walrus's --enable-ldw-opt
    (redundant load-weight elimination) run, which is incompatible with
    standalone Ldweights instructions."""
    n = 0
    for blk in nc.m.functions[0].blocks:
        insts = list(blk.instructions)
        keep = []
        pending = None
        for inst in insts:
            if isinstance(inst, mybir.InstLdweights):
                assert pending is None, "consecutive Ldweights"
                pending = inst
                continue
            if pending is not None and str(inst.engine).endswith("PE"):
                assert isinstance(inst, mybir.InstMatmult), (
                    f"LDW followed by PE {inst.concise_opcode()}")
                si = pending.sync_info
                if si is not None and (len(si.on_wait) or len(si.on_update)):
                    mi = inst.sync_info
                    ow = list(si.on_wait) + (list(mi.on_wait) if mi else [])
                    ou = list(si.on_update) + (list(mi.on_update) if mi else [])
                    inst.sync_info = mybir.SyncInfo(on_wait=ow, on_update=ou)
                inst.ldweights = True
                pending = None
                n += 1
            keep.append(inst)
        assert pending is None, "trailing Ldweights"
        blk.instructions.clear()
        for i in keep:
            blk.add_instruction(i)
    return n


def build_bass(TG=2, FG=4, repeat=1, chunk_t=512, w_transpose="pe",
               xq_engine="sync", oq_engine="gpsimd", wq_engine="scalar",
               mm_only=False, x_bufs=2, ps_bufs=4, out_bufs=2,
               f_inner=True, wprep_bufs=2, nch_cap=0, fuse_ldw=False,
               wk_split=1, wprep_eng="vector", wcopy_alt=False,
               w_layout="rowmajor", wstage_bufs=3):
    T_c, F_c, K = T // TG, OUT // FG, IN
    nc = bacc.Bacc("TRN2", target_bir_lowering=False, debug=False,
                   enable_asserts=False, num_devices=1)

    KT = K // P            # 32 k-tiles
    WTILES = F_c // P      # weight row tiles
    FB = F_c // 512        # psum-wide output blocks
    NCH = T_c // chunk_t   # token chunks
    TPC = chunk_t // P     # token tiles per chunk
    if nch_cap:
        NCH = min(NCH, nch_cap)

    xt_ap = nc.dram_tensor("xt", [(T_c // chunk_t) * P, KT, chunk_t], BF16,
                           kind="ExternalInput").ap()
    if w_layout == "kmajor":
        base_ap = nc.dram_tensor("base", [KT * P, F_c], BF16,
                                 kind="ExternalInput").ap()
        signs_ap = nc.dram_tensor("signs", [KT * P, F_c], U8,
                                  kind="ExternalInput").ap()
        scales_ap = nc.dram_tensor("scales", [P, F_c], F32,
                                   kind="ExternalInput").ap()
    else:
        base_ap = nc.dram_tensor("base", [F_c, K], BF16,
                                 kind="ExternalInput").ap()
        signs_ap = nc.dram_tensor("signs", [F_c, K], U8,
                                  kind="ExternalInput").ap()
        scales_ap = nc.dram_tensor("scales", [F_c], F32,
                                   kind="ExternalInput").ap()
    out_ap = nc.dram_tensor("out", [T_c, F_c], F32, kind="ExternalOutput").ap()

    with tile.TileContext(nc) as tc:
        with ExitStack() as ctx:
            const = ctx.enter_context(tc.tile_pool(name="const", bufs=1))
            wprep = ctx.enter_context(tc.tile_pool(name="wprep",
                                                   bufs=wprep_bufs))
            wtp = ctx.enter_context(tc.tile_pool(name="wtp", bufs=1))
            xcp = ctx.enter_context(tc.tile_pool(name="xcp", bufs=x_bufs))
            outp = ctx.enter_context(tc.tile_pool(name="outp", bufs=out_bufs))
            ps_t = (ctx.enter_context(
                tc.tile_pool(name="ps_t", bufs=2, space="PSUM"))
                if (w_transpose == "pe" and w_layout != "kmajor")
                else None)
            ps_c = ctx.enter_context(
                tc.tile_pool(name="ps_c", bufs=ps_bufs, space="PSUM"))

            if repeat > 1:
                ctx.enter_context(tc.For_i(0, repeat, 1))

            # resident k-major weights: [128, KT, F_c] bf16
            wt_sb = wtp.tile([P, KT, F_c], BF16)

            if w_layout == "kmajor":
                # scales arrive replicated [128, F_c]; per-k-tile delta
                # apply with plain tensor_tensor ops (no PE, no PSUM).
                scB = const.tile([P, F_c], F32)
                nc.sync.dma_start(scB[:], scales_ap[:, :])
                scB2 = const.tile([P, F_c], F32)
                nc.vector.tensor_scalar_mul(scB2[:], scB[:], 2.0)
                for kt in range(KT):
                    bt = wprep.tile([P, F_c], BF16, tag="wbase")
                    getattr(nc, wq_engine).dma_start(
                        bt[:], base_ap[kt * P:(kt + 1) * P, :])
                    st = wprep.tile([P, F_c], U8, tag="wsigns")
                    getattr(nc, wq_engine).dma_start(
                        st[:], signs_ap[kt * P:(kt + 1) * P, :])
                    t1 = wprep.tile([P, F_c], BF16, tag="wt1")
                    nc.vector.tensor_sub(t1[:], bt[:], scB[:])
                    t2 = wprep.tile([P, F_c], BF16, tag="wt2")
                    nc.gpsimd.tensor_mul(t2[:], st[:], scB2[:])
                    eng = nc.vector if kt % 2 == 0 else nc.gpsimd
                    eng.tensor_add(wt_sb[:, kt, :], t1[:], t2[:])

            if w_layout != "kmajor":
                ident = const.tile([P, P], BF16)
                make_identity(nc, ident)

                # scales [F_c] -> SBUF [128, WTILES]; f = wt*128 + p
                sc = const.tile([P, WTILES], F32)
                nc.sync.dma_start(sc[:],
                                  scales_ap.rearrange("(a p) -> p a", p=P))
                sc2 = const.tile([P, WTILES], F32)
                nc.vector.tensor_scalar_mul(sc2[:], sc[:], 2.0)

            # ---- Phase W: delta-apply + transpose weights ----
            KS = K // wk_split
            for wt in range(WTILES if w_layout != "kmajor" else 0):
                for wc in range(wk_split):
                    c0 = wc * KS
                    bt = wprep.tile([P, KS], BF16, tag="wbase")
                    getattr(nc, wq_engine).dma_start(
                        bt[:], base_ap[wt * P:(wt + 1) * P, c0:c0 + KS])
                    st = wprep.tile([P, KS], U8, tag="wsigns")
                    getattr(nc, wq_engine).dma_start(
                        st[:], signs_ap[wt * P:(wt + 1) * P, c0:c0 + KS])

                    # b2 = base - scale (per-partition scalar)
                    weng = getattr(nc, wprep_eng)
                    b2 = wprep.tile([P, KS], BF16, tag="wb2")
                    weng.tensor_scalar_sub(b2[:], bt[:], sc[:, wt:wt + 1])
                    # w = signs * (2*scale) + b2 -> bf16
                    wbf = wprep.tile([P, KS], BF16, tag="wbf")
                    weng.scalar_tensor_tensor(
                        out=wbf[:], in0=st[:], scalar=sc2[:, wt:wt + 1],
                        in1=b2[:], op0=AluOpType.mult, op1=AluOpType.add)

                    if w_transpose == "dma":
                        nc.scalar.dma_start(
                            wt_sb[:, c0 // P:(c0 + KS) // P,
                                  wt * P:(wt + 1) * P],
                            wbf[:], transpose=True)
                    else:
                        for kg in range(KS // 512):
                            ps = ps_t.tile([P, 512], F32, tag="ps_t")
                            for j in range(4):
                                kt = kg * 4 + j
                                nc.tensor.matmul(
                                    ps[:, j * P:(j + 1) * P],
                                    wbf[:, kt * P:(kt + 1) * P], ident[:])
                            wdst = wt_sb[:, c0 // P + kg * 4:
                                         c0 // P + (kg + 1) * 4,
                                         wt * P:(wt + 1) * P]
                            wsrc = ps.rearrange("p (a b) -> p a b", b=P)
                            if wcopy_alt and kg % 2 == 0:
                                nc.scalar.copy(out=wdst, in_=wsrc)
                            else:
                                nc.vector.tensor_copy(out=wdst, in_=wsrc)

            # ---- Phase C: stream token chunks; pure MM pipeline ----
            for c in range(NCH):
                xc = xcp.tile([P, KT, chunk_t], BF16, tag="xc", name="xc")
                if mm_only:
                    if c == 0:
                        nc.vector.memset(xc[:, 0, 0:1], 0.0)
                else:
                    getattr(nc, xq_engine).dma_start(
                        xc[:], xt_ap[c * P:(c + 1) * P])

                for tt in range(TPC):
                    lo = tt * P
                    pcs = [ps_c.tile([P, 512], F32, tag="ps_c", name=f"pc{f}")
                           for f in range(FB)]
                    if f_inner:
                        for k in range(KT):
                            for f in range(FB):
                                nc.tensor.matmul(
                                    pcs[f][:], xc[:, k, lo:lo + P],
                                    wt_sb[:, k, f * 512:(f + 1) * 512],
                                    start=(k == 0), stop=(k == KT - 1))
                    else:
                        for f in range(FB):
                            for k in range(KT):
                                nc.tensor.matmul(
                                    pcs[f][:], xc[:, k, lo:lo + P],
                                    wt_sb[:, k, f * 512:(f + 1) * 512],
                                    start=(k == 0), stop=(k == KT - 1))

                    ot = outp.tile([P, F_c], F32, name="ot")
                    for f in range(FB):
                        if f % 2 == 0:
                            nc.scalar.copy(out=ot[:, f * 512:(f + 1) * 512],
                                           in_=pcs[f][:])
                        else:
                            nc.vector.tensor_copy(
                                out=ot[:, f * 512:(f + 1) * 512],
                                in_=pcs[f][:])
                    t0 = c * chunk_t + tt * P
                    getattr(nc, oq_engine).dma_start(
                        out_ap[t0:t0 + P, :], ot[:])

    if fuse_ldw:
        _fuse_ldweights(nc)
    nc.compile()
    return nc


class SpmdRunner:
    """Builds the sharded jitted callable once (mirrors
    concourse.bass2jax.run_bass_via_pjrt's multi-core branch) so repeated
    executions skip re-tracing and reuse the cached NEFF."""

    def __init__(self, nc, n_cores):
        import jax
        from jax.sharding import Mesh, PartitionSpec
        from jax.experimental.shard_map import shard_map
        from concourse.bass2jax import (
            _bass_exec_p, install_neuronx_cc_hook, partition_id_tensor)

        self.jax = jax
        self.PartitionSpec = PartitionSpec
        install_neuronx_cc_hook()
        assert nc.dbg_addr is None
        self.n_cores = n_cores
        partition_name = (
            nc.partition_id_tensor.name if nc.partition_id_tensor else None)
        in_names, out_names, out_avals, zero_outs = [], [], [], []
        for alloc in nc.m.functions[0].allocations:
            if not isinstance(alloc, mybir.MemoryLocationSet):
                continue
            name = alloc.memorylocations[0].name
            if alloc.kind == "ExternalInput":
                if name != partition_name:
                    in_names.append(name)
            elif alloc.kind == "ExternalOutput":
                shape = tuple(alloc.tensor_shape)
                dtype = mybir.dt.np(alloc.dtype)
                out_names.append(name)
                out_avals.append(jax.core.ShapedArray(shape, dtype))
                zero_outs.append(np.zeros(shape, dtype))
        n_params = len(in_names)
        n_outs = len(out_avals)
        full_in_names = list(in_names) + list(out_names)
        if partition_name is not None:
            full_in_names.append(partition_name)
        self.in_names = in_names
        self.out_names = out_names
        self.out_avals = out_avals
        self.zero_outs = zero_outs

        def _body(*args):
            operands = list(args)
            if partition_name is not None:
                operands.append(partition_id_tensor())
            outs = _bass_exec_p.bind(
                *operands,
                out_avals=tuple(out_avals),
                in_names=tuple(full_in_names),
                out_names=tuple(out_names),
                lowering_input_output_aliases=(),
                sim_require_finite=True,
                sim_require_nnan=True,
                nc=nc,
            )
            return tuple(outs)

        devices = jax.devices()[:n_cores]
        assert len(devices) == n_cores, (
            f"need {n_cores} cores, have {len(jax.devices())}")
        mesh = Mesh(np.asarray(devices), ("core",))
        in_specs = (PartitionSpec("core"),) * (n_params + n_outs)
        out_specs = (PartitionSpec("core"),) * n_outs
        donate = tuple(range(n_params, n_params + n_outs))
        self.sharded = jax.jit(
            shard_map(_body, mesh=mesh, in_specs=in_specs,
                      out_specs=out_specs, check_rep=False),
            donate_argnums=donate, keep_unused=True)
        self.mesh = mesh

    def prep_inputs(self, in_maps):
        from jax.sharding import NamedSharding

        sh = NamedSharding(self.mesh, self.PartitionSpec("core"))
        concat = [
            np.concatenate([np.asarray(in_maps[c][name])
                            for c in range(self.n_cores)], axis=0)
            for name in self.in_names
        ]
        out = [self.jax.device_put(a, sh) for a in concat]
        self.jax.block_until_ready(out)
        return out

    def zeros(self):
        import jax.numpy as jnp
        from jax.sharding import NamedSharding

        if not hasattr(self, "_zeros_fn"):
            shardings = tuple(
                NamedSharding(self.mesh, self.PartitionSpec("core"))
                for _ in self.zero_outs)
            shapes = [((self.n_cores * z.shape[0], *z.shape[1:]), z.dtype)
                      for z in self.zero_outs]
            self._zeros_fn = self.jax.jit(
                lambda: tuple(jnp.zeros(s, d) for s, d in shapes),
                out_shardings=shardings)
        out = self._zeros_fn()
        self.jax.block_until_ready(out)
        return list(out)

    def __call__(self, prepped_inputs, zeros=None):
        if zeros is None:
            zeros = self.zeros()
        out_arrs = self.sharded(*prepped_inputs, *zeros)
        self.jax.block_until_ready(out_arrs)
        return out_arrs

    def split_outputs(self, out_arrs):
        return [
            {name: np.asarray(out_arrs[i]).reshape(
                self.n_cores, *self.out_avals[i].shape)[c]
             for i, name in enumerate(self.out_names)}
            for c in range(self.n_cores)
        ]




CONFIG = dict(fuse_ldw=True, w_layout="kmajor", ps_bufs=6, x_bufs=3)


_CACHE = {}


def _get_runner(repeat=1, **cfg):
    key = (repeat, tuple(sorted(cfg.items())))
    if key not in _CACHE:
        nc = build_bass(repeat=repeat, **{**CONFIG, **cfg})
        _CACHE[key] = SpmdRunner(nc, N_CORES)
    return _CACHE[key]


def _relayout_x(Xg, chunk_t=512):
    """[T_c, K] f32 -> [(T_c//chunk_t)*128, KT, chunk_t] bf16 contiguous."""
    T_c, K = Xg.shape
    xt = Xg.reshape(T_c // chunk_t, chunk_t, K // P, P)
    xt = np.ascontiguousarray(xt.transpose(0, 3, 2, 1))
    return xt.astype(ml_dtypes.bfloat16).reshape(-1, K // P, chunk_t)


def _shard_inputs(input, base_weight, delta_signs, delta_scales,
                  TG=2, FG=4, chunk_t=512, w_layout="kmajor"):
    T_C, F_C = T // TG, OUT // FG
    X = np.asarray(input, dtype=np.float32).reshape(T, IN)
    base = np.asarray(base_weight, dtype=np.float32)
    signs = np.asarray(delta_signs)
    scales = np.asarray(delta_scales, dtype=np.float32)
    xts = [_relayout_x(X[tg * T_C:(tg + 1) * T_C], chunk_t)
           for tg in range(TG)]
    in_maps = []
    for c in range(N_CORES):
        tg, fg = divmod(c, FG)
        bc = base[fg * F_C:(fg + 1) * F_C]
        sg = signs[fg * F_C:(fg + 1) * F_C]
        sl = scales[fg * F_C:(fg + 1) * F_C]
        if w_layout == "kmajor":
            in_maps.append({
                "xt": xts[tg],
                "base": np.ascontiguousarray(bc.T).astype(ml_dtypes.bfloat16),
                "signs": np.ascontiguousarray(sg.T).astype(np.uint8),
                "scales": np.ascontiguousarray(
                    np.broadcast_to(sl, (P, F_C))),
            })
        else:
            in_maps.append({
                "xt": xts[tg],
                "base": bc.astype(ml_dtypes.bfloat16),
                "signs": sg.astype(np.uint8),
                "scales": sl,
            })
    return in_maps


def kernel(input, base_weight, delta_signs, delta_scales):
    TG, FG = 2, 4
    T_C, F_C = T // TG, OUT // FG
    runner = _get_runner()
    in_maps = _shard_inputs(input, base_weight, delta_signs, delta_scales,
                            TG=TG, FG=FG)
    prepped = runner.prep_inputs(in_maps)
    out_arrs = runner(prepped)
    res = runner.split_outputs(out_arrs)
    C = np.empty((T, OUT), np.float32)
    for c in range(N_CORES):
        tg, fg = divmod(c, FG)
        C[tg * T_C:(tg + 1) * T_C, fg * F_C:(fg + 1) * F_C] = res[c]["out"]
    return C.reshape(B, S, OUT)


# revision 4
# speedup vs baseline: 1.0626x; 1.0626x over previous
"""BitDelta linear on 8 TRN2 NeuronCores — v2.

C[b,s,o] = sum_i X[b,s,i] * (base[o,i] + (2*signs[o,i]-1)*scales[o])

Sharding: TG token-groups x FG feature-groups. The host side casts X to
bf16 and re-lays it out i-major (X^T) so the device does zero X
transposes; each token chunk is one contiguous DMA (32KB/partition
descriptors):

  xt[(c*128+p), k, t] = X[tg*T_C + c*chunk_t + t, k*128 + p]   (bf16)

On device the weight delta W = base + (2s-1)*scale is applied on DVE
(base arrives bf16, signs uint8 {0,1}, scales f32), W^T is built
k-major via identity-matmul on the PE once (startup only), and the
main loop is a pure MM stream: per 128-token tile, KT ldweights of the
stationary X^T tile + KT*FB matmuls of N=512 accumulating into PSUM,
then ACT/DVE copy PSUM->SBUF and DMA out.

Optionally flips walrus's --enable-ldw-opt to true (redundant
load-weight elimination) via LDW_OPT.
"""

import sys

sys.path.insert(0, "/opt/trn_rl_repo")

from contextlib import ExitStack

import numpy as np
import ml_dtypes

import concourse.bass as bass
import concourse.tile as tile
import concourse.bass_utils as _bu
from concourse import bacc, mybir
from concourse.alu_op_type import AluOpType
from concourse.masks import make_identity

F32 = mybir.dt.float32
BF16 = mybir.dt.bfloat16
U8 = mybir.dt.uint8
P = 128

B, S, IN, OUT = 8, 2048, 4096, 4096
T = B * S
N_CORES = 8

# ---- walrus flag patch: redundant-ldweights elimination ----
LDW_OPT = False
_orig_run_command = _bu.run_command


def _run_command_ldw(cmd, **kw):
    if LDW_OPT and isinstance(cmd, list):
        cmd = ["--enable-ldw-opt=true" if c == "--enable-ldw-opt=false" else c
               for c in cmd]
    return _orig_run_command(cmd, **kw)


_bu.run_command = _run_command_ldw


def _fuse_ldweights(nc):
    """Fold standalone InstLdweights into their InstMatmult (self-loading),
    transferring sync waits/updates. Lets walrus's --enable-ldw-opt
    (redundant load-weight elimination) run, which is incompatible with
    standalone Ldweights instructions."""
    n = 0
    for blk in nc.m.functions[0].blocks:
        insts = list(blk.instructions)
        keep = []
        pending = None
        for inst in insts:
            if isinstance(inst, mybir.InstLdweights):
                assert pending is None, "consecutive Ldweights"
                pending = inst
                continue
            if pending is not None and str(inst.engine).endswith("PE"):
                assert isinstance(inst, mybir.InstMatmult), (
                    f"LDW followed by PE {inst.concise_opcode()}")
                si = pending.sync_info
                if si is not None and (len(si.on_wait) or len(si.on_update)):
                    mi = inst.sync_info
                    ow = list(si.on_wait) + (list(mi.on_wait) if mi else [])
                    ou = list(si.on_update) + (list(mi.on_update) if mi else [])
                    inst.sync_info = mybir.SyncInfo(on_wait=ow, on_update=ou)
                inst.ldweights = True
                pending = None
                n += 1
            keep.append(inst)
        assert pending is None, "trailing Ldweights"
        blk.instructions.clear()
        for i in keep:
            blk.add_instruction(i)
    return n


def build_bass(TG=2, FG=4, repeat=1, chunk_t=512, w_transpose="pe",
               xq_engine="sync", oq_engine="gpsimd", wq_engine="scalar",
               mm_only=False, x_bufs=2, ps_bufs=4, out_bufs=2,
               f_inner=True, wprep_bufs=2, nch_cap=0, fuse_ldw=False,
               wk_split=1, wprep_eng="vector", wcopy_alt=False,
               w_layout="rowmajor", wstage_bufs=3, ham_keepalive=False,
               wq2_engine=None):
    T_c, F_c, K = T // TG, OUT // FG, IN
    nc = bacc.Bacc("TRN2", target_bir_lowering=False, debug=False,
                   enable_asserts=False, num_devices=1)

    KT = K // P            # 32 k-tiles
    WTILES = F_c // P      # weight row tiles
    FB = F_c // 512        # psum-wide output blocks
    NCH = T_c // chunk_t   # token chunks
    TPC = chunk_t // P     # token tiles per chunk
    if nch_cap:
        NCH = min(NCH, nch_cap)

    xt_ap = nc.dram_tensor("xt", [(T_c // chunk_t) * P, KT, chunk_t], BF16,
                           kind="ExternalInput").ap()
    if w_layout == "kmajor":
        base_ap = nc.dram_tensor("base", [KT * P, F_c], BF16,
                                 kind="ExternalInput").ap()
        signs_ap = nc.dram_tensor("signs", [KT * P, F_c], U8,
                                  kind="ExternalInput").ap()
        scales_ap = nc.dram_tensor("scales", [P, F_c], F32,
                                   kind="ExternalInput").ap()
    else:
        base_ap = nc.dram_tensor("base", [F_c, K], BF16,
                                 kind="ExternalInput").ap()
        signs_ap = nc.dram_tensor("signs", [F_c, K], U8,
                                  kind="ExternalInput").ap()
        scales_ap = nc.dram_tensor("scales", [F_c], F32,
                                   kind="ExternalInput").ap()
    out_ap = nc.dram_tensor("out", [T_c, F_c], F32, kind="ExternalOutput").ap()

    with tile.TileContext(nc) as tc:
        with ExitStack() as ctx:
            const = ctx.enter_context(tc.tile_pool(name="const", bufs=1))
            wprep = ctx.enter_context(tc.tile_pool(name="wprep",
                                                   bufs=wprep_bufs))
            wtp = ctx.enter_context(tc.tile_pool(name="wtp", bufs=1))
            xcp = ctx.enter_context(tc.tile_pool(name="xcp", bufs=x_bufs))
            outp = ctx.enter_context(tc.tile_pool(name="outp", bufs=out_bufs))
            ps_t = (ctx.enter_context(
                tc.tile_pool(name="ps_t", bufs=2, space="PSUM"))
                if (w_transpose == "pe" and w_layout != "kmajor")
                else None)
            ps_c = ctx.enter_context(
                tc.tile_pool(name="ps_c", bufs=ps_bufs, space="PSUM"))

            if repeat > 1:
                ctx.enter_context(tc.For_i(0, repeat, 1))

            # resident k-major weights: [128, KT, F_c] bf16
            wt_sb = wtp.tile([P, KT, F_c], BF16)

            if w_layout == "kmajor":
                # scales arrive replicated [128, F_c]; per-k-tile delta
                # apply with plain tensor_tensor ops (no PE, no PSUM).
                scB = const.tile([P, F_c], F32)
                nc.sync.dma_start(scB[:], scales_ap[:, :])
                scB2 = const.tile([P, F_c], F32)
                nc.vector.tensor_scalar_mul(scB2[:], scB[:], 2.0)
                wq2 = wq2_engine or wq_engine
                for kt in range(KT):
                    bt = wprep.tile([P, F_c], BF16, tag="wbase")
                    getattr(nc, wq_engine).dma_start(
                        bt[:], base_ap[kt * P:(kt + 1) * P, :])
                    st = wprep.tile([P, F_c], U8, tag="wsigns")
                    getattr(nc, wq2).dma_start(
                        st[:], signs_ap[kt * P:(kt + 1) * P, :])
                    t1 = wprep.tile([P, F_c], BF16, tag="wt1")
                    nc.vector.tensor_sub(t1[:], bt[:], scB[:])
                    t2 = wprep.tile([P, F_c], BF16, tag="wt2")
                    nc.gpsimd.tensor_mul(t2[:], st[:], scB2[:])
                    eng = nc.vector if kt % 2 == 0 else nc.gpsimd
                    eng.tensor_add(wt_sb[:, kt, :], t1[:], t2[:])
                    if ham_keepalive and kt % 2 == 1:
                        # dummy MM on freshly written W keeps the PE HAM
                        # clock-gate warm through the (PE-idle) W phase
                        ps_k = ps_c.tile([P, 512], F32, tag="ps_c",
                                         name="keep")
                        nc.tensor.matmul(ps_k[:], wt_sb[:, kt, 0:P],
                                         wt_sb[:, kt, 0:512])

            if w_layout != "kmajor":
                ident = const.tile([P, P], BF16)
                make_identity(nc, ident)

                # scales [F_c] -> SBUF [128, WTILES]; f = wt*128 + p
                sc = const.tile([P, WTILES], F32)
                nc.sync.dma_start(sc[:],
                                  scales_ap.rearrange("(a p) -> p a", p=P))
                sc2 = const.tile([P, WTILES], F32)
                nc.vector.tensor_scalar_mul(sc2[:], sc[:], 2.0)

            # ---- Phase W: delta-apply + transpose weights ----
            KS = K // wk_split
            for wt in range(WTILES if w_layout != "kmajor" else 0):
                for wc in range(wk_split):
                    c0 = wc * KS
                    bt = wprep.tile([P, KS], BF16, tag="wbase")
                    getattr(nc, wq_engine).dma_start(
                        bt[:], base_ap[wt * P:(wt + 1) * P, c0:c0 + KS])
                    st = wprep.tile([P, KS], U8, tag="wsigns")
                    getattr(nc, wq_engine).dma_start(
                        st[:], signs_ap[wt * P:(wt + 1) * P, c0:c0 + KS])

                    # b2 = base - scale (per-partition scalar)
                    weng = getattr(nc, wprep_eng)
                    b2 = wprep.tile([P, KS], BF16, tag="wb2")
                    weng.tensor_scalar_sub(b2[:], bt[:], sc[:, wt:wt + 1])
                    # w = signs * (2*scale) + b2 -> bf16
                    wbf = wprep.tile([P, KS], BF16, tag="wbf")
                    weng.scalar_tensor_tensor(
                        out=wbf[:], in0=st[:], scalar=sc2[:, wt:wt + 1],
                        in1=b2[:], op0=AluOpType.mult, op1=AluOpType.add)

                    if w_transpose == "dma":
                        nc.scalar.dma_start(
                            wt_sb[:, c0 // P:(c0 + KS) // P,
                                  wt * P:(wt + 1) * P],
                            wbf[:], transpose=True)
                    else:
                        for kg in range(KS // 512):
                            ps = ps_t.tile([P, 512], F32, tag="ps_t")
                            for j in range(4):
                                kt = kg * 4 + j
                                nc.tensor.matmul(
                                    ps[:, j * P:(j + 1) * P],
                                    wbf[:, kt * P:(kt + 1) * P], ident[:])
                            wdst = wt_sb[:, c0 // P + kg * 4:
                                         c0 // P + (kg + 1) * 4,
                                         wt * P:(wt + 1) * P]
                            wsrc = ps.rearrange("p (a b) -> p a b", b=P)
                            if wcopy_alt and kg % 2 == 0:
                                nc.scalar.copy(out=wdst, in_=wsrc)
                            else:
                                nc.vector.tensor_copy(out=wdst, in_=wsrc)

            # ---- Phase C: stream token chunks; pure MM pipeline ----
            for c in range(NCH):
                xc = xcp.tile([P, KT, chunk_t], BF16, tag="xc", name="xc")
                if mm_only:
                    if c == 0:
                        nc.vector.memset(xc[:, 0, 0:1], 0.0)
                else:
                    getattr(nc, xq_engine).dma_start(
                        xc[:], xt_ap[c * P:(c + 1) * P])

                for tt in range(TPC):
                    lo = tt * P
                    pcs = [ps_c.tile([P, 512], F32, tag="ps_c", name=f"pc{f}")
                           for f in range(FB)]
                    if f_inner:
                        for k in range(KT):
                            for f in range(FB):
                                nc.tensor.matmul(
                                    pcs[f][:], xc[:, k, lo:lo + P],
                                    wt_sb[:, k, f * 512:(f + 1) * 512],
                                    start=(k == 0), stop=(k == KT - 1))
                    else:
                        for f in range(FB):
                            for k in range(KT):
                                nc.tensor.matmul(
                                    pcs[f][:], xc[:, k, lo:lo + P],
                                    wt_sb[:, k, f * 512:(f + 1) * 512],
                                    start=(k == 0), stop=(k == KT - 1))

                    ot = outp.tile([P, F_c], F32, name="ot")
                    for f in range(FB):
                        if f % 2 == 0:
                            nc.scalar.copy(out=ot[:, f * 512:(f + 1) * 512],
                                           in_=pcs[f][:])
                        else:
                            nc.vector.tensor_copy(
                                out=ot[:, f * 512:(f + 1) * 512],
                                in_=pcs[f][:])
                    t0 = c * chunk_t + tt * P
                    getattr(nc, oq_engine).dma_start(
                        out_ap[t0:t0 + P, :], ot[:])

    if fuse_ldw:
        _fuse_ldweights(nc)
    nc.compile()
    return nc


class SpmdRunner:
    """Builds the sharded jitted callable once (mirrors
    concourse.bass2jax.run_bass_via_pjrt's multi-core branch) so repeated
    executions skip re-tracing and reuse the cached NEFF."""

    def __init__(self, nc, n_cores):
        import jax
        from jax.sharding import Mesh, PartitionSpec
        from jax.experimental.shard_map import shard_map
        from concourse.bass2jax import (
            _bass_exec_p, install_neuronx_cc_hook, partition_id_tensor)

        self.jax = jax
        self.PartitionSpec = PartitionSpec
        install_neuronx_cc_hook()
        assert nc.dbg_addr is None
        self.n_cores = n_cores
        partition_name = (
            nc.partition_id_tensor.name if nc.partition_id_tensor else None)
        in_names, out_names, out_avals, zero_outs = [], [], [], []
        for alloc in nc.m.functions[0].allocations:
            if not isinstance(alloc, mybir.MemoryLocationSet):
                continue
            name = alloc.memorylocations[0].name
            if alloc.kind == "ExternalInput":
                if name != partition_name:
                    in_names.append(name)
            elif alloc.kind == "ExternalOutput":
                shape = tuple(alloc.tensor_shape)
                dtype = mybir.dt.np(alloc.dtype)
                out_names.append(name)
                out_avals.append(jax.core.ShapedArray(shape, dtype))
                zero_outs.append(np.zeros(shape, dtype))
        n_params = len(in_names)
        n_outs = len(out_avals)
        full_in_names = list(in_names) + list(out_names)
        if partition_name is not None:
            full_in_names.append(partition_name)
        self.in_names = in_names
        self.out_names = out_names
        self.out_avals = out_avals
        self.zero_outs = zero_outs

        def _body(*args):
            operands = list(args)
            if partition_name is not None:
                operands.append(partition_id_tensor())
            outs = _bass_exec_p.bind(
                *operands,
                out_avals=tuple(out_avals),
                in_names=tuple(full_in_names),
                out_names=tuple(out_names),
                lowering_input_output_aliases=(),
                sim_require_finite=True,
                sim_require_nnan=True,
                nc=nc,
            )
            return tuple(outs)

        devices = jax.devices()[:n_cores]
        assert len(devices) == n_cores, (
            f"need {n_cores} cores, have {len(jax.devices())}")
        mesh = Mesh(np.asarray(devices), ("core",))
        in_specs = (PartitionSpec("core"),) * (n_params + n_outs)
        out_specs = (PartitionSpec("core"),) * n_outs
        donate = tuple(range(n_params, n_params + n_outs))
        self.sharded = jax.jit(
            shard_map(_body, mesh=mesh, in_specs=in_specs,
                      out_specs=out_specs, check_rep=False),
            donate_argnums=donate, keep_unused=True)
        self.mesh = mesh

    def prep_inputs(self, in_maps):
        from jax.sharding import NamedSharding

        sh = NamedSharding(self.mesh, self.PartitionSpec("core"))
        concat = [
            np.concatenate([np.asarray(in_maps[c][name])
                            for c in range(self.n_cores)], axis=0)
            for name in self.in_names
        ]
        out = [self.jax.device_put(a, sh) for a in concat]
        self.jax.block_until_ready(out)
        return out

    def zeros(self):
        import jax.numpy as jnp
        from jax.sharding import NamedSharding

        if not hasattr(self, "_zeros_fn"):
            shardings = tuple(
                NamedSharding(self.mesh, self.PartitionSpec("core"))
                for _ in self.zero_outs)
            shapes = [((self.n_cores * z.shape[0], *z.shape[1:]), z.dtype)
                      for z in self.zero_outs]
            self._zeros_fn = self.jax.jit(
                lambda: tuple(jnp.zeros(s, d) for s, d in shapes),
                out_shardings=shardings)
        out = self._zeros_fn()
        self.jax.block_until_ready(out)
        return list(out)

    def __call__(self, prepped_inputs, zeros=None):
        if zeros is None:
            zeros = self.zeros()
        out_arrs = self.sharded(*prepped_inputs, *zeros)
        self.jax.block_until_ready(out_arrs)
        return out_arrs

    def split_outputs(self, out_arrs):
        return [
            {name: np.asarray(out_arrs[i]).reshape(
                self.n_cores, *self.out_avals[i].shape)[c]
             for i, name in enumerate(self.out_names)}
            for c in range(self.n_cores)
        ]




CONFIG = dict(fuse_ldw=True, w_layout="kmajor", ps_bufs=6, x_bufs=3,
              ham_keepalive=True, wq2_engine="gpsimd")


_CACHE = {}


def _get_runner(repeat=1, **cfg):
    key = (repeat, tuple(sorted(cfg.items())))
    if key not in _CACHE:
        nc = build_bass(repeat=repeat, **{**CONFIG, **cfg})
        _CACHE[key] = SpmdRunner(nc, N_CORES)
    return _CACHE[key]


def _relayout_x(Xg, chunk_t=512):
    """[T_c, K] f32 -> [(T_c//chunk_t)*128, KT, chunk_t] bf16 contiguous."""
    T_c, K = Xg.shape
    xt = Xg.reshape(T_c // chunk_t, chunk_t, K // P, P)
    xt = np.ascontiguousarray(xt.transpose(0, 3, 2, 1))
    return xt.astype(ml_dtypes.bfloat16).reshape(-1, K // P, chunk_t)


def _shard_inputs(input, base_weight, delta_signs, delta_scales,
                  TG=2, FG=4, chunk_t=512, w_layout="kmajor"):
    T_C, F_C = T // TG, OUT // FG
    X = np.asarray(input, dtype=np.float32).reshape(T, IN)
    base = np.asarray(base_weight, dtype=np.float32)
    signs = np.asarray(delta_signs)
    scales = np.asarray(delta_scales, dtype=np.float32)
    xts = [_relayout_x(X[tg * T_C:(tg + 1) * T_C], chunk_t)
           for tg in range(TG)]
    in_maps = []
    for c in range(N_CORES):
        tg, fg = divmod(c, FG)
        bc = base[fg * F_C:(fg + 1) * F_C]
        sg = signs[fg * F_C:(fg + 1) * F_C]
        sl = scales[fg * F_C:(fg + 1) * F_C]
        if w_layout == "kmajor":
            in_maps.append({
                "xt": xts[tg],
                "base": np.ascontiguousarray(bc.T).astype(ml_dtypes.bfloat16),
                "signs": np.ascontiguousarray(sg.T).astype(np.uint8),
                "scales": np.ascontiguousarray(
                    np.broadcast_to(sl, (P, F_C))),
            })
        else:
            in_maps.append({
                "xt": xts[tg],
                "base": bc.astype(ml_dtypes.bfloat16),
                "signs": sg.astype(np.uint8),
                "scales": sl,
            })
    return in_maps


def kernel(input, base_weight, delta_signs, delta_scales):
    TG, FG = 2, 4
    T_C, F_C = T // TG, OUT // FG
    runner = _get_runner()
    in_maps = _shard_inputs(input, base_weight, delta_signs, delta_scales,
                            TG=TG, FG=FG)
    prepped = runner.prep_inputs(in_maps)
    out_arrs = runner(prepped)
    res = runner.split_outputs(out_arrs)
    C = np.empty((T, OUT), np.float32)
    for c in range(N_CORES):
        tg, fg = divmod(c, FG)
        C[tg * T_C:(tg + 1) * T_C, fg * F_C:(fg + 1) * F_C] = res[c]["out"]
    return C.reshape(B, S, OUT)
